# revision 1
# baseline (speedup 1.0000x reference)
"""Location-sensitive attention TRN2 Bass kernel.

Data-parallel over batch: B=64 sharded as 8 per NeuronCore across 8 cores;
parameters replicated. Per core:

  query   = decoder_hidden @ Wq                     [8, 128]   (prep, on PE)
  keys    = encoder_outputs @ Wk                    [8, 2048, 128]
  loc     = conv1d(prev_attention) ; loc_term = loc @ Wl
  energy  = tanh(keys + query + loc_term) @ v       [8, 2048]
  out     = softmax(energy, axis=T)

Design notes (measured on HW):
 * enc arrives [tok, feat] with feat contiguous; the PE matmul contracts over
   the partition dim, so enc is transposed on-chip. The transposes are issued
   as REGULAR fp16 matmuls against an identity (out = lhsT.T @ I), not
   transpose-mode ops: transpose-mode does not register as PE activity in the
   HAM clock monitor, which leaves the whole PE stream throttled at 1.2 GHz
   (measured 4.8x slower end-to-end).
 * fp16 operands: 1 cyc/row matmul rate (4x faster than fp32, same as bf16)
   with a 10-bit mantissa (~1e-3 rel err vs ~1e-2 for bf16).
 * enc is loaded in 2MB-read chunks (1024 tokens), partition p holding 8
   consecutive tokens (8KB contiguous HBM per partition): measured 422 GB/s
   vs 217 GB/s for the partition-strided layout. Token order inside a group
   becomes t = 8p + s; softmax sums are order-invariant and the final
   normalization multiply un-permutes via its read access pattern.
 * enc loads ride SWDGE (gpsimd; f32->fp16 cast in the DMA). Small DMAs
   (exp-row assembly, output) ride the sync HWDGE ring so neither queue
   head-of-line blocks the other.
 * conv+Wl+conv_b fold into one [32, 128] matrix WW (rows 0..30 =
   sum_c conv_w[c,k]*Wl[c,:], row 31 = conv_b @ Wl), applied against a
   [32, 2048] shifted-window view of prev_attention (row 31 = ones) and
   accumulated into the same PSUM tile as the keys matmuls. query is the
   per-partition bias of the tanh activation. exp needs no max-subtraction:
   |energy| <= ||v||_1 ~ 11, safely inside fp32 exp range.
"""
import sys

sys.path.insert(0, "/opt/trn_rl_repo")

from contextlib import ExitStack

import numpy as np

import concourse.bass as bass
import concourse.tile as tile
from concourse import bacc, mybir
from concourse.bass_utils import run_bass_kernel_spmd
from concourse.masks import make_identity

B, T, ENC_DIM = 64, 2048, 512
Q_DIM, ATTN, CH, KS, PAD = 256, 128, 32, 31, 15
N_CORES = 8
BL = B // N_CORES  # 8 batches per core

f32 = mybir.dt.float32
fp16 = mybir.dt.float16
AF = mybir.ActivationFunctionType


def build(reps: int = 1):
    nc = bacc.Bacc("TRN2", target_bir_lowering=False, debug=False,
                   num_devices=N_CORES)

    enc_d = nc.dram_tensor("encoder_outputs", [BL, T, ENC_DIM], f32,
                           kind="ExternalInput").ap()
    dh_d = nc.dram_tensor("decoder_hidden", [BL, Q_DIM], f32,
                          kind="ExternalInput").ap()
    pa_d = nc.dram_tensor("prev_attention", [BL, T], f32,
                          kind="ExternalInput").ap()
    wq_d = nc.dram_tensor("Wq", [Q_DIM, ATTN], f32, kind="ExternalInput").ap()
    wk_d = nc.dram_tensor("Wk", [ENC_DIM, ATTN], f32, kind="ExternalInput").ap()
    cw_d = nc.dram_tensor("conv_w", [CH, 1, KS], f32, kind="ExternalInput").ap()
    cb_d = nc.dram_tensor("conv_b", [CH], f32, kind="ExternalInput").ap()
    wl_d = nc.dram_tensor("Wl", [CH, ATTN], f32, kind="ExternalInput").ap()
    v_d = nc.dram_tensor("v", [ATTN], f32, kind="ExternalInput").ap()
    out_d = nc.dram_tensor("out", [BL, T], f32, kind="ExternalOutput").ap()

    # internal DRAM scratch for the zero-padded prev_attention rows
    pa_pad_d = nc.dram_tensor("pa_pad", [BL, T + 32], f32).ap()

    with tile.TileContext(nc) as tc, ExitStack() as ctx:
        singles = ctx.enter_context(tc.tile_pool(name="singles", bufs=1))
        sb_enc = ctx.enter_context(tc.tile_pool(name="enc", bufs=4))
        sb_xt = ctx.enter_context(tc.tile_pool(name="xt", bufs=4))
        sb_tanh = ctx.enter_context(tc.tile_pool(name="tanh", bufs=3))
        sb_sm = ctx.enter_context(tc.tile_pool(name="sm", bufs=2))
        ps_xt = ctx.enter_context(tc.tile_pool(name="ps_xt", bufs=3, space="PSUM"))
        ps_o = ctx.enter_context(tc.tile_pool(name="ps_o", bufs=4, space="PSUM"))
        ps_prep = ctx.enter_context(tc.tile_pool(name="ps_prep", bufs=1, space="PSUM"))

        # ---------------- constants ----------------
        ident_f = singles.tile([128, 128], f32)
        make_identity(nc, ident_f)
        ident = singles.tile([128, 128], fp16)
        nc.vector.tensor_copy(ident, ident_f)

        wk_sb = singles.tile([128, 4, ATTN], fp16)
        nc.gpsimd.dma_start(wk_sb, wk_d.rearrange("(c k) a -> k c a", c=4))
        wq_sb = singles.tile([128, 2, ATTN], fp16)
        nc.gpsimd.dma_start(wq_sb, wq_d.rearrange("(c k) a -> k c a", c=2))
        dh_sb = singles.tile([BL, Q_DIM], fp16)
        nc.gpsimd.dma_start(dh_sb, dh_d)
        cwb_sb = singles.tile([CH, 32], fp16)
        nc.gpsimd.dma_start(cwb_sb[:, 0:KS], cw_d.rearrange("c o k -> c (o k)"))
        nc.gpsimd.dma_start(
            cwb_sb[:, KS:KS + 1],
            bass.AP(tensor=cb_d.tensor, offset=0, ap=[[1, CH], [1, 1]]))
        wl_sb = singles.tile([CH, ATTN], fp16)
        nc.gpsimd.dma_start(wl_sb, wl_d)
        v_sb = singles.tile([ATTN, 1], fp16)
        nc.gpsimd.dma_start(
            v_sb, bass.AP(tensor=v_d.tensor, offset=0, ap=[[1, ATTN], [1, 1]]))

        # ---------------- prep: queryT, WW ----------------
        # dhT [256, 8] via two transpose-matmuls of dh [8, 256]
        dhT_ps = ps_prep.tile([128, 2, BL], f32, tag="prep")
        for c in range(2):
            nc.tensor.matmul(dhT_ps[:, c, :], dh_sb[:, c * 128:(c + 1) * 128],
                             ident[0:BL, 0:BL], start=True, stop=True)
        dhT_sb = singles.tile([128, 2, BL], fp16)
        nc.vector.tensor_copy(dhT_sb, dhT_ps)

        # queryT [A, 8] = Wq.T @ dhT  (accumulate 2 chunks of q-dim)
        qt_ps = ps_prep.tile([ATTN, BL], f32, tag="prep")
        for c in range(2):
            nc.tensor.matmul(qt_ps, wq_sb[:, c, :], dhT_sb[:, c, :],
                             start=(c == 0), stop=(c == 1))
        qt_sb = singles.tile([ATTN, BL], f32)
        nc.scalar.copy(qt_sb, qt_ps)

        # WW [32, A]: rows 0..30 = sum_c conv_w[c,k] Wl[c,:], row 31 = conv_b @ Wl
        ww_ps = ps_prep.tile([32, ATTN], f32, tag="prep")
        nc.tensor.matmul(ww_ps, cwb_sb, wl_sb, start=True, stop=True)
        ww_sb = singles.tile([32, ATTN], fp16)
        nc.vector.tensor_copy(ww_sb, ww_ps)

        # ---------------- prep: shifted prev_attention windows ----------------
        pa_stage = singles.tile([BL, T + 32], f32)
        nc.vector.memset(pa_stage, 0.0)
        nc.sync.dma_start(pa_stage[:, PAD:PAD + T], pa_d)
        nc.sync.dma_start(pa_pad_d, pa_stage)

        ones_sb = singles.tile([1, T], f32)
        nc.vector.memset(ones_sb, 1.0)
        ones_d = nc.dram_tensor("ones_row", [T], f32).ap()
        nc.sync.dma_start(ones_d, ones_sb)

        # pa_sh[k, b, t] = pa_pad[b, t + k]  (k=0..30), row 31 = ones
        pa_sh = singles.tile([32, BL, T], fp16)
        nc.gpsimd.dma_start(
            pa_sh[0:KS, :, :],
            bass.AP(tensor=pa_pad_d.tensor, offset=0,
                    ap=[[1, KS], [T + 32, BL], [1, T]]))
        nc.gpsimd.dma_start(
            pa_sh[KS:KS + 1, :, :],
            bass.AP(tensor=ones_d.tensor, offset=0,
                    ap=[[0, 1], [0, BL], [1, T]]))

        # ---------------- main loop ----------------
        # supertile = 1024 tokens; partition p holds tokens 8p..8p+7 of it.
        NSUP = T // 1024  # 2 per batch row
        for rep in range(reps):
            exp_sb = sb_sm.tile([BL, T], f32, tag="exp")
            for b in range(BL):
                for G in range(NSUP):
                    enc_sb = sb_enc.tile([128, 8, ENC_DIM], fp16, tag="enc")
                    nc.gpsimd.dma_start(
                        enc_sb,
                        enc_d[b, G * 1024:(G + 1) * 1024, :]
                        .rearrange("(p s) f -> p s f", p=128))
                    for h in range(2):
                        out_ps = ps_o.tile([ATTN, 512], f32, tag="o")
                        for c in range(4):
                            xt_ps = ps_xt.tile([128, 512], f32, tag="xt")
                            for q in range(4):
                                nc.tensor.matmul(
                                    xt_ps[:, q * 128:(q + 1) * 128],
                                    enc_sb[:, 4 * h + q, c * 128:(c + 1) * 128],
                                    ident, start=True, stop=True)
                            xt_sb = sb_xt.tile([128, 512], fp16, tag="xts")
                            if c == 3:
                                nc.scalar.copy(xt_sb, xt_ps)
                            else:
                                nc.vector.tensor_copy(xt_sb, xt_ps)
                            nc.tensor.matmul(out_ps, wk_sb[:, c, :], xt_sb,
                                             start=(c == 0), stop=False)
                        # loc term: pa columns in permuted token order
                        # col j=(q,p) -> token 8p + 4h + q of this supertile
                        _sl = pa_sh[:, b, G * 1024 + 4 * h:]
                        pa_slice = bass.AP(tensor=_sl.tensor, offset=_sl.offset,
                                           ap=[_sl.ap[0], [1, 4], [8, 128]])
                        nc.tensor.matmul(out_ps, ww_sb, pa_slice,
                                         start=False, stop=True)

                        tanh_sb = sb_tanh.tile([ATTN, 512], fp16, tag="tanh")
                        nc.scalar.activation(tanh_sb, out_ps, AF.Tanh,
                                             bias=qt_sb[:, b:b + 1])

                        e_ps = ps_o.tile([1, 512], f32, tag="o")
                        nc.tensor.matmul(e_ps, v_sb, tanh_sb, start=True,
                                         stop=True)
                        exp_g = sb_xt.tile([1, 512], f32, tag="expg")
                        nc.scalar.activation(exp_g, e_ps, AF.Exp)
                        # ACT cannot write at partition base b; HWDGE DMA can
                        # (sync ring: keeps it off the SWDGE enc stream).
                        nc.sync.dma_start(
                            exp_sb[b:b + 1,
                                   G * 1024 + h * 512:G * 1024 + (h + 1) * 512],
                            exp_g)

            # softmax normalization over T, batched across the 8 rows
            sums = sb_sm.tile([BL, 1], f32, tag="sums")
            nc.vector.reduce_sum(sums, exp_sb, axis=mybir.AxisListType.X)
            inv = sb_sm.tile([BL, 1], f32, tag="inv")
            nc.vector.reciprocal(inv, sums)
            o_sb = sb_sm.tile([BL, T], f32, tag="osb")
            # un-permute: natural token t = G*1024 + 8p + 4h + q reads storage
            # index G*1024 + h*512 + q*128 + p
            _e = exp_sb[:, :]
            exp_perm = bass.AP(
                tensor=_e.tensor, offset=_e.offset,
                ap=[_e.ap[0], [1024, NSUP], [1, 128], [512, 2], [128, 4]])
            nc.vector.tensor_scalar_mul(o_sb, exp_perm, inv)
            nc.sync.dma_start(out_d, o_sb)

    nc.compile()
    return nc


_cache = {}


def _get(reps: int = 1):
    if reps not in _cache:
        _cache[reps] = build(reps)
    return _cache[reps]


def _in_maps(inputs):
    enc = np.ascontiguousarray(np.asarray(inputs["encoder_outputs"], dtype=np.float32))
    dh = np.ascontiguousarray(np.asarray(inputs["decoder_hidden"], dtype=np.float32))
    pa = np.ascontiguousarray(np.asarray(inputs["prev_attention"], dtype=np.float32))
    rep = {k: np.ascontiguousarray(np.asarray(inputs[k], dtype=np.float32))
           for k in ("Wq", "Wk", "conv_w", "conv_b", "Wl", "v")}
    maps = []
    for i in range(N_CORES):
        s = slice(i * BL, (i + 1) * BL)
        maps.append({"encoder_outputs": enc[s], "decoder_hidden": dh[s],
                     "prev_attention": pa[s], **rep})
    return maps


def kernel(**inputs) -> np.ndarray:
    nc = _get(1)
    res = run_bass_kernel_spmd(nc, _in_maps(inputs), list(range(N_CORES)))
    return np.concatenate([res.results[i]["out"] for i in range(N_CORES)],
                          axis=0).astype(np.float32)


if __name__ == "__main__":
    rng = np.random.default_rng(0)
    ins = {
        "encoder_outputs": rng.standard_normal((B, T, ENC_DIM), dtype=np.float32),
        "decoder_hidden": rng.standard_normal((B, Q_DIM), dtype=np.float32),
        "prev_attention": rng.random((B, T), dtype=np.float32),
        "Wq": (rng.standard_normal((Q_DIM, ATTN), dtype=np.float32) / np.sqrt(Q_DIM)),
        "Wk": (rng.standard_normal((ENC_DIM, ATTN), dtype=np.float32) / np.sqrt(ENC_DIM)),
        "conv_w": (rng.standard_normal((CH, 1, KS), dtype=np.float32) / np.sqrt(KS)),
        "conv_b": np.zeros(CH, dtype=np.float32),
        "Wl": (rng.standard_normal((CH, ATTN), dtype=np.float32) / np.sqrt(CH)),
        "v": (rng.standard_normal(ATTN, dtype=np.float32) / np.sqrt(ATTN)),
    }
    out = kernel(**ins)
    print("kernel output", out.shape, out.dtype, "row sums ~1:",
          np.allclose(out.sum(axis=1), 1.0, atol=1e-3))



# revision 2
# speedup vs baseline: 15.9277x; 15.9277x over previous
"""Location-sensitive attention TRN2 Bass kernel.

Data-parallel over batch: B=64 sharded as 8 per NeuronCore across 8 cores;
parameters replicated. Per core:

  query   = decoder_hidden @ Wq                     [8, 128]   (prep, on PE)
  keys    = encoder_outputs @ Wk                    [8, 2048, 128]
  loc     = conv1d(prev_attention) ; loc_term = loc @ Wl
  energy  = tanh(keys + query + loc_term) @ v       [8, 2048]
  out     = softmax(energy, axis=T)

Design notes (measured on HW):
 * enc arrives [tok, feat] with feat contiguous; the PE matmul contracts over
   the partition dim, so enc is transposed on-chip. The transposes are issued
   as REGULAR fp16 matmuls against an identity (out = lhsT.T @ I), not
   transpose-mode ops: transpose-mode does not register as PE activity in the
   HAM clock monitor, which leaves the whole PE stream throttled at 1.2 GHz
   (measured 4.8x slower end-to-end).
 * fp16 operands: 1 cyc/row matmul rate (4x faster than fp32, same as bf16)
   with a 10-bit mantissa (~1e-3 rel err vs ~1e-2 for bf16).
 * enc is loaded in 2MB-read chunks (1024 tokens), partition p holding 8
   consecutive tokens (8KB contiguous HBM per partition): measured 422 GB/s
   vs 217 GB/s for the partition-strided layout. Token order inside a group
   becomes t = 8p + s; softmax sums are order-invariant and the final
   normalization multiply un-permutes via its read access pattern.
 * enc loads ride SWDGE (gpsimd; f32->fp16 cast in the DMA). Small DMAs
   (exp-row assembly, output) ride the sync HWDGE ring so neither queue
   head-of-line blocks the other.
 * pool buffer counts are tuned (ps_xt=4, ps_o=3, sb_xt=6, sb_tanh=4):
   deepening the transpose-PSUM rotation while narrowing the keys-PSUM
   rotation removes a recurring ~308ns/h-iter PE stall at the c-chunk
   handoff (TimelineSim steady state 81.4us -> 71.5us per rep).
 * conv+Wl+conv_b fold into one [32, 128] matrix WW (rows 0..30 =
   sum_c conv_w[c,k]*Wl[c,:], row 31 = conv_b @ Wl), applied against a
   [32, 2048] shifted-window view of prev_attention (row 31 = ones) and
   accumulated into the same PSUM tile as the keys matmuls. query is the
   per-partition bias of the tanh activation. exp needs no max-subtraction:
   |energy| <= ||v||_1 ~ 11, safely inside fp32 exp range.
"""
import sys

sys.path.insert(0, "/opt/trn_rl_repo")

from contextlib import ExitStack

import numpy as np

import concourse.bass as bass
import concourse.tile as tile
from concourse import bacc, mybir
from concourse.bass_utils import run_bass_kernel_spmd
from concourse.masks import make_identity

B, T, ENC_DIM = 64, 2048, 512
Q_DIM, ATTN, CH, KS, PAD = 256, 128, 32, 31, 15
N_CORES = 8
BL = B // N_CORES  # 8 batches per core

f32 = mybir.dt.float32
fp16 = mybir.dt.float16
AF = mybir.ActivationFunctionType


def build(reps: int = 1):
    nc = bacc.Bacc("TRN2", target_bir_lowering=False, debug=False,
                   num_devices=N_CORES)

    enc_d = nc.dram_tensor("encoder_outputs", [BL, T, ENC_DIM], f32,
                           kind="ExternalInput").ap()
    dh_d = nc.dram_tensor("decoder_hidden", [BL, Q_DIM], f32,
                          kind="ExternalInput").ap()
    pa_d = nc.dram_tensor("prev_attention", [BL, T], f32,
                          kind="ExternalInput").ap()
    wq_d = nc.dram_tensor("Wq", [Q_DIM, ATTN], f32, kind="ExternalInput").ap()
    wk_d = nc.dram_tensor("Wk", [ENC_DIM, ATTN], f32, kind="ExternalInput").ap()
    cw_d = nc.dram_tensor("conv_w", [CH, 1, KS], f32, kind="ExternalInput").ap()
    cb_d = nc.dram_tensor("conv_b", [CH], f32, kind="ExternalInput").ap()
    wl_d = nc.dram_tensor("Wl", [CH, ATTN], f32, kind="ExternalInput").ap()
    v_d = nc.dram_tensor("v", [ATTN], f32, kind="ExternalInput").ap()
    out_d = nc.dram_tensor("out", [BL, T], f32, kind="ExternalOutput").ap()

    # internal DRAM scratch for the zero-padded prev_attention rows
    pa_pad_d = nc.dram_tensor("pa_pad", [BL, T + 32], f32).ap()

    with tile.TileContext(nc) as tc, ExitStack() as ctx:
        singles = ctx.enter_context(tc.tile_pool(name="singles", bufs=1))
        sb_enc = ctx.enter_context(tc.tile_pool(name="enc", bufs=4))
        sb_xt = ctx.enter_context(tc.tile_pool(name="xt", bufs=6))
        sb_tanh = ctx.enter_context(tc.tile_pool(name="tanh", bufs=4))
        sb_sm = ctx.enter_context(tc.tile_pool(name="sm", bufs=2))
        ps_xt = ctx.enter_context(tc.tile_pool(name="ps_xt", bufs=4, space="PSUM"))
        ps_o = ctx.enter_context(tc.tile_pool(name="ps_o", bufs=3, space="PSUM"))
        ps_prep = ctx.enter_context(tc.tile_pool(name="ps_prep", bufs=1, space="PSUM"))

        # ---------------- constants ----------------
        ident_f = singles.tile([128, 128], f32)
        make_identity(nc, ident_f)
        ident = singles.tile([128, 128], fp16)
        nc.vector.tensor_copy(ident, ident_f)

        wk_sb = singles.tile([128, 4, ATTN], fp16)
        nc.gpsimd.dma_start(wk_sb, wk_d.rearrange("(c k) a -> k c a", c=4))
        wq_sb = singles.tile([128, 2, ATTN], fp16)
        nc.gpsimd.dma_start(wq_sb, wq_d.rearrange("(c k) a -> k c a", c=2))
        dh_sb = singles.tile([BL, Q_DIM], fp16)
        nc.gpsimd.dma_start(dh_sb, dh_d)
        cwb_sb = singles.tile([CH, 32], fp16)
        nc.gpsimd.dma_start(cwb_sb[:, 0:KS], cw_d.rearrange("c o k -> c (o k)"))
        nc.gpsimd.dma_start(
            cwb_sb[:, KS:KS + 1],
            bass.AP(tensor=cb_d.tensor, offset=0, ap=[[1, CH], [1, 1]]))
        wl_sb = singles.tile([CH, ATTN], fp16)
        nc.gpsimd.dma_start(wl_sb, wl_d)
        v_sb = singles.tile([ATTN, 1], fp16)
        nc.gpsimd.dma_start(
            v_sb, bass.AP(tensor=v_d.tensor, offset=0, ap=[[1, ATTN], [1, 1]]))

        # ---------------- prep: queryT, WW ----------------
        # dhT [256, 8] via two transpose-matmuls of dh [8, 256]
        dhT_ps = ps_prep.tile([128, 2, BL], f32, tag="prep")
        for c in range(2):
            nc.tensor.matmul(dhT_ps[:, c, :], dh_sb[:, c * 128:(c + 1) * 128],
                             ident[0:BL, 0:BL], start=True, stop=True)
        dhT_sb = singles.tile([128, 2, BL], fp16)
        nc.vector.tensor_copy(dhT_sb, dhT_ps)

        # queryT [A, 8] = Wq.T @ dhT  (accumulate 2 chunks of q-dim)
        qt_ps = ps_prep.tile([ATTN, BL], f32, tag="prep")
        for c in range(2):
            nc.tensor.matmul(qt_ps, wq_sb[:, c, :], dhT_sb[:, c, :],
                             start=(c == 0), stop=(c == 1))
        qt_sb = singles.tile([ATTN, BL], f32)
        nc.scalar.copy(qt_sb, qt_ps)

        # WW [32, A]: rows 0..30 = sum_c conv_w[c,k] Wl[c,:], row 31 = conv_b @ Wl
        ww_ps = ps_prep.tile([32, ATTN], f32, tag="prep")
        nc.tensor.matmul(ww_ps, cwb_sb, wl_sb, start=True, stop=True)
        ww_sb = singles.tile([32, ATTN], fp16)
        nc.vector.tensor_copy(ww_sb, ww_ps)

        # ---------------- prep: shifted prev_attention windows ----------------
        pa_stage = singles.tile([BL, T + 32], f32)
        nc.vector.memset(pa_stage, 0.0)
        nc.sync.dma_start(pa_stage[:, PAD:PAD + T], pa_d)
        nc.sync.dma_start(pa_pad_d, pa_stage)

        ones_sb = singles.tile([1, T], f32)
        nc.vector.memset(ones_sb, 1.0)
        ones_d = nc.dram_tensor("ones_row", [T], f32).ap()
        nc.sync.dma_start(ones_d, ones_sb)

        # pa_sh[k, b, t] = pa_pad[b, t + k]  (k=0..30), row 31 = ones
        pa_sh = singles.tile([32, BL, T], fp16)
        nc.gpsimd.dma_start(
            pa_sh[0:KS, :, :],
            bass.AP(tensor=pa_pad_d.tensor, offset=0,
                    ap=[[1, KS], [T + 32, BL], [1, T]]))
        nc.gpsimd.dma_start(
            pa_sh[KS:KS + 1, :, :],
            bass.AP(tensor=ones_d.tensor, offset=0,
                    ap=[[0, 1], [0, BL], [1, T]]))

        # ---------------- main loop ----------------
        # supertile = 1024 tokens; partition p holds tokens 8p..8p+7 of it.
        NSUP = T // 1024  # 2 per batch row
        for rep in range(reps):
            exp_sb = sb_sm.tile([BL, T], f32, tag="exp")
            for b in range(BL):
                for G in range(NSUP):
                    enc_sb = sb_enc.tile([128, 8, ENC_DIM], fp16, tag="enc")
                    nc.gpsimd.dma_start(
                        enc_sb,
                        enc_d[b, G * 1024:(G + 1) * 1024, :]
                        .rearrange("(p s) f -> p s f", p=128))
                    for h in range(2):
                        out_ps = ps_o.tile([ATTN, 512], f32, tag="o")
                        for c in range(4):
                            xt_ps = ps_xt.tile([128, 512], f32, tag="xt")
                            for q in range(4):
                                nc.tensor.matmul(
                                    xt_ps[:, q * 128:(q + 1) * 128],
                                    enc_sb[:, 4 * h + q, c * 128:(c + 1) * 128],
                                    ident, start=True, stop=True)
                            xt_sb = sb_xt.tile([128, 512], fp16, tag="xts")
                            if c == 3:
                                nc.scalar.copy(xt_sb, xt_ps)
                            else:
                                nc.vector.tensor_copy(xt_sb, xt_ps)
                            nc.tensor.matmul(out_ps, wk_sb[:, c, :], xt_sb,
                                             start=(c == 0), stop=False)
                        # loc term: pa columns in permuted token order
                        # col j=(q,p) -> token 8p + 4h + q of this supertile
                        _sl = pa_sh[:, b, G * 1024 + 4 * h:]
                        pa_slice = bass.AP(tensor=_sl.tensor, offset=_sl.offset,
                                           ap=[_sl.ap[0], [1, 4], [8, 128]])
                        nc.tensor.matmul(out_ps, ww_sb, pa_slice,
                                         start=False, stop=True)

                        tanh_sb = sb_tanh.tile([ATTN, 512], fp16, tag="tanh")
                        nc.scalar.activation(tanh_sb, out_ps, AF.Tanh,
                                             bias=qt_sb[:, b:b + 1])

                        e_ps = ps_o.tile([1, 512], f32, tag="o")
                        nc.tensor.matmul(e_ps, v_sb, tanh_sb, start=True,
                                         stop=True)
                        exp_g = sb_xt.tile([1, 512], f32, tag="expg")
                        nc.scalar.activation(exp_g, e_ps, AF.Exp)
                        # ACT cannot write at partition base b; HWDGE DMA can
                        # (sync ring: keeps it off the SWDGE enc stream).
                        nc.sync.dma_start(
                            exp_sb[b:b + 1,
                                   G * 1024 + h * 512:G * 1024 + (h + 1) * 512],
                            exp_g)

            # softmax normalization over T, batched across the 8 rows
            sums = sb_sm.tile([BL, 1], f32, tag="sums")
            nc.vector.reduce_sum(sums, exp_sb, axis=mybir.AxisListType.X)
            inv = sb_sm.tile([BL, 1], f32, tag="inv")
            nc.vector.reciprocal(inv, sums)
            o_sb = sb_sm.tile([BL, T], f32, tag="osb")
            # un-permute: natural token t = G*1024 + 8p + 4h + q reads storage
            # index G*1024 + h*512 + q*128 + p
            _e = exp_sb[:, :]
            exp_perm = bass.AP(
                tensor=_e.tensor, offset=_e.offset,
                ap=[_e.ap[0], [1024, NSUP], [1, 128], [512, 2], [128, 4]])
            nc.vector.tensor_scalar_mul(o_sb, exp_perm, inv)
            nc.sync.dma_start(out_d, o_sb)

    nc.compile()
    return nc


_cache = {}


def _get(reps: int = 1):
    if reps not in _cache:
        _cache[reps] = build(reps)
    return _cache[reps]


def _in_maps(inputs):
    enc = np.ascontiguousarray(np.asarray(inputs["encoder_outputs"], dtype=np.float32))
    dh = np.ascontiguousarray(np.asarray(inputs["decoder_hidden"], dtype=np.float32))
    pa = np.ascontiguousarray(np.asarray(inputs["prev_attention"], dtype=np.float32))
    rep = {k: np.ascontiguousarray(np.asarray(inputs[k], dtype=np.float32))
           for k in ("Wq", "Wk", "conv_w", "conv_b", "Wl", "v")}
    maps = []
    for i in range(N_CORES):
        s = slice(i * BL, (i + 1) * BL)
        maps.append({"encoder_outputs": enc[s], "decoder_hidden": dh[s],
                     "prev_attention": pa[s], **rep})
    return maps


def kernel(**inputs) -> np.ndarray:
    nc = _get(1)
    res = run_bass_kernel_spmd(nc, _in_maps(inputs), list(range(N_CORES)))
    return np.concatenate([res.results[i]["out"] for i in range(N_CORES)],
                          axis=0).astype(np.float32)


if __name__ == "__main__":
    rng = np.random.default_rng(0)
    ins = {
        "encoder_outputs": rng.standard_normal((B, T, ENC_DIM), dtype=np.float32),
        "decoder_hidden": rng.standard_normal((B, Q_DIM), dtype=np.float32),
        "prev_attention": rng.random((B, T), dtype=np.float32),
        "Wq": (rng.standard_normal((Q_DIM, ATTN), dtype=np.float32) / np.sqrt(Q_DIM)),
        "Wk": (rng.standard_normal((ENC_DIM, ATTN), dtype=np.float32) / np.sqrt(ENC_DIM)),
        "conv_w": (rng.standard_normal((CH, 1, KS), dtype=np.float32) / np.sqrt(KS)),
        "conv_b": np.zeros(CH, dtype=np.float32),
        "Wl": (rng.standard_normal((CH, ATTN), dtype=np.float32) / np.sqrt(CH)),
        "v": (rng.standard_normal(ATTN, dtype=np.float32) / np.sqrt(ATTN)),
    }
    out = kernel(**ins)
    print("kernel output", out.shape, out.dtype, "row sums ~1:",
          np.allclose(out.sum(axis=1), 1.0, atol=1e-3))



# revision 3
# speedup vs baseline: 17.3415x; 1.0888x over previous
"""Location-sensitive attention TRN2 Bass kernel.

Data-parallel over batch: B=64 sharded as 8 per NeuronCore across 8 cores;
parameters replicated. Per core:

  query   = decoder_hidden @ Wq                     [8, 128]   (prep, on PE)
  keys    = encoder_outputs @ Wk                    [8, 2048, 128]
  loc     = conv1d(prev_attention) ; loc_term = loc @ Wl
  energy  = tanh(keys + query + loc_term) @ v       [8, 2048]
  out     = softmax(energy, axis=T)

Design notes (measured on HW):
 * enc arrives [tok, feat] with feat contiguous; the PE matmul contracts over
   the partition dim, so enc is transposed on-chip. The transposes are issued
   as REGULAR fp16 matmuls against an identity (out = lhsT.T @ I), not
   transpose-mode ops: transpose-mode does not register as PE activity in the
   HAM clock monitor, which leaves the whole PE stream throttled at 1.2 GHz
   (measured 4.8x slower end-to-end).
 * fp16 operands: 1 cyc/row matmul rate (4x faster than fp32, same as bf16)
   with a 10-bit mantissa (~1e-3 rel err vs ~1e-2 for bf16).
 * enc is loaded in 2MB-read chunks (1024 tokens), partition p holding 8
   consecutive tokens (8KB contiguous HBM per partition): measured 422 GB/s
   vs 217 GB/s for the partition-strided layout. Token order inside a group
   becomes t = 8p + s; softmax sums are order-invariant and the final
   normalization multiply un-permutes via its read access pattern.
 * enc loads ride SWDGE (gpsimd; f32->fp16 cast in the DMA). Small DMAs
   (exp-row assembly, output) ride the sync HWDGE ring so neither queue
   head-of-line blocks the other.
 * pool buffer counts are tuned (ps_xt=4, ps_o=3, sb_xt=6, sb_tanh=4):
   deepening the transpose-PSUM rotation while narrowing the keys-PSUM
   rotation removes a recurring ~308ns/h-iter PE stall at the c-chunk
   handoff (TimelineSim steady state 81.4us -> 71.5us per rep).
 * conv+Wl+conv_b fold into one [32, 128] matrix WW (rows 0..30 =
   sum_c conv_w[c,k]*Wl[c,:], row 31 = conv_b @ Wl), applied against a
   [32, 2048] shifted-window view of prev_attention (row 31 = ones) and
   accumulated into the same PSUM tile as the keys matmuls. query is the
   per-partition bias of the tanh activation. exp needs no max-subtraction:
   |energy| <= ||v||_1 ~ 11, safely inside fp32 exp range.
"""
import sys

sys.path.insert(0, "/opt/trn_rl_repo")

from contextlib import ExitStack

import numpy as np

import concourse.bass as bass
import concourse.tile as tile
from concourse import bacc, mybir
from concourse.bass_utils import run_bass_kernel_spmd
from concourse.masks import make_identity

B, T, ENC_DIM = 64, 2048, 512
Q_DIM, ATTN, CH, KS, PAD = 256, 128, 32, 31, 15
N_CORES = 8
BL = B // N_CORES  # 8 batches per core

f32 = mybir.dt.float32
fp16 = mybir.dt.float16
AF = mybir.ActivationFunctionType


def build(reps: int = 1):
    nc = bacc.Bacc("TRN2", target_bir_lowering=False, debug=False,
                   num_devices=N_CORES)

    enc_d = nc.dram_tensor("encoder_outputs", [BL, T, ENC_DIM], f32,
                           kind="ExternalInput").ap()
    dh_d = nc.dram_tensor("decoder_hidden", [BL, Q_DIM], f32,
                          kind="ExternalInput").ap()
    pa_d = nc.dram_tensor("prev_attention", [BL, T], f32,
                          kind="ExternalInput").ap()
    wq_d = nc.dram_tensor("Wq", [Q_DIM, ATTN], f32, kind="ExternalInput").ap()
    wk_d = nc.dram_tensor("Wk", [ENC_DIM, ATTN], f32, kind="ExternalInput").ap()
    cw_d = nc.dram_tensor("conv_w", [CH, 1, KS], f32, kind="ExternalInput").ap()
    cb_d = nc.dram_tensor("conv_b", [CH], f32, kind="ExternalInput").ap()
    wl_d = nc.dram_tensor("Wl", [CH, ATTN], f32, kind="ExternalInput").ap()
    v_d = nc.dram_tensor("v", [ATTN], f32, kind="ExternalInput").ap()
    out_d = nc.dram_tensor("out", [BL, T], f32, kind="ExternalOutput").ap()

    # internal DRAM scratch for the zero-padded prev_attention rows
    pa_pad_d = nc.dram_tensor("pa_pad", [BL, T + 32], f32).ap()

    with tile.TileContext(nc) as tc, ExitStack() as ctx:
        singles = ctx.enter_context(tc.tile_pool(name="singles", bufs=1))
        sb_enc = ctx.enter_context(tc.tile_pool(name="enc", bufs=6))
        sb_xt = ctx.enter_context(tc.tile_pool(name="xt", bufs=6))
        sb_tanh = ctx.enter_context(tc.tile_pool(name="tanh", bufs=4))
        sb_sm = ctx.enter_context(tc.tile_pool(name="sm", bufs=3))
        ps_xt = ctx.enter_context(tc.tile_pool(name="ps_xt", bufs=4, space="PSUM"))
        ps_o = ctx.enter_context(tc.tile_pool(name="ps_o", bufs=3, space="PSUM"))
        ps_prep = ctx.enter_context(tc.tile_pool(name="ps_prep", bufs=1, space="PSUM"))

        # ---------------- constants ----------------
        ident_f = singles.tile([128, 128], f32)
        make_identity(nc, ident_f)
        ident = singles.tile([128, 128], fp16)
        nc.vector.tensor_copy(ident, ident_f)

        wk_sb = singles.tile([128, 4, ATTN], fp16)
        nc.gpsimd.dma_start(wk_sb, wk_d.rearrange("(c k) a -> k c a", c=4))
        wq_sb = singles.tile([128, 2, ATTN], fp16)
        nc.gpsimd.dma_start(wq_sb, wq_d.rearrange("(c k) a -> k c a", c=2))
        dh_sb = singles.tile([BL, Q_DIM], fp16)
        nc.gpsimd.dma_start(dh_sb, dh_d)
        cwb_sb = singles.tile([CH, 32], fp16)
        nc.gpsimd.dma_start(cwb_sb[:, 0:KS], cw_d.rearrange("c o k -> c (o k)"))
        nc.gpsimd.dma_start(
            cwb_sb[:, KS:KS + 1],
            bass.AP(tensor=cb_d.tensor, offset=0, ap=[[1, CH], [1, 1]]))
        wl_sb = singles.tile([CH, ATTN], fp16)
        nc.gpsimd.dma_start(wl_sb, wl_d)
        v_sb = singles.tile([ATTN, 1], fp16)
        nc.gpsimd.dma_start(
            v_sb, bass.AP(tensor=v_d.tensor, offset=0, ap=[[1, ATTN], [1, 1]]))

        # ---------------- prep: queryT, WW ----------------
        # dhT [256, 8] via two transpose-matmuls of dh [8, 256]
        dhT_ps = ps_prep.tile([128, 2, BL], f32, tag="prep")
        for c in range(2):
            nc.tensor.matmul(dhT_ps[:, c, :], dh_sb[:, c * 128:(c + 1) * 128],
                             ident[0:BL, 0:BL], start=True, stop=True)
        dhT_sb = singles.tile([128, 2, BL], fp16)
        nc.vector.tensor_copy(dhT_sb, dhT_ps)

        # queryT [A, 8] = Wq.T @ dhT  (accumulate 2 chunks of q-dim)
        qt_ps = ps_prep.tile([ATTN, BL], f32, tag="prep")
        for c in range(2):
            nc.tensor.matmul(qt_ps, wq_sb[:, c, :], dhT_sb[:, c, :],
                             start=(c == 0), stop=(c == 1))
        qt_sb = singles.tile([ATTN, BL], f32)
        nc.scalar.copy(qt_sb, qt_ps)

        # WW [32, A]: rows 0..30 = sum_c conv_w[c,k] Wl[c,:], row 31 = conv_b @ Wl
        ww_ps = ps_prep.tile([32, ATTN], f32, tag="prep")
        nc.tensor.matmul(ww_ps, cwb_sb, wl_sb, start=True, stop=True)
        ww_sb = singles.tile([32, ATTN], fp16)
        nc.vector.tensor_copy(ww_sb, ww_ps)

        # ---------------- prep: shifted prev_attention windows ----------------
        pa_stage = singles.tile([BL, T + 32], f32)
        nc.vector.memset(pa_stage, 0.0)
        nc.sync.dma_start(pa_stage[:, PAD:PAD + T], pa_d)
        nc.sync.dma_start(pa_pad_d, pa_stage)

        ones_sb = singles.tile([1, T], f32)
        nc.vector.memset(ones_sb, 1.0)
        ones_d = nc.dram_tensor("ones_row", [T], f32).ap()
        nc.sync.dma_start(ones_d, ones_sb)

        # pa_sh[k, b, t] = pa_pad[b, t + k]  (k=0..30), row 31 = ones
        pa_sh = singles.tile([32, BL, T], fp16)
        nc.gpsimd.dma_start(
            pa_sh[0:KS, :, :],
            bass.AP(tensor=pa_pad_d.tensor, offset=0,
                    ap=[[1, KS], [T + 32, BL], [1, T]]))
        nc.gpsimd.dma_start(
            pa_sh[KS:KS + 1, :, :],
            bass.AP(tensor=ones_d.tensor, offset=0,
                    ap=[[0, 1], [0, BL], [1, T]]))

        # ---------------- main loop ----------------
        # supertile = 1024 tokens; partition p holds tokens 8p..8p+7 of it.
        NSUP = T // 1024  # 2 per batch row
        for rep in range(reps):
            exp_sb = sb_sm.tile([BL, T], f32, tag="exp")
            for b in range(BL):
                for G in range(NSUP):
                    enc_sb = sb_enc.tile([128, 8, ENC_DIM], fp16, tag="enc")
                    nc.gpsimd.dma_start(
                        enc_sb,
                        enc_d[b, G * 1024:(G + 1) * 1024, :]
                        .rearrange("(p s) f -> p s f", p=128))
                    for h in range(2):
                        out_ps = ps_o.tile([ATTN, 512], f32, tag="o")
                        for c in range(4):
                            xt_ps = ps_xt.tile([128, 512], f32, tag="xt")
                            for q in range(4):
                                nc.tensor.matmul(
                                    xt_ps[:, q * 128:(q + 1) * 128],
                                    enc_sb[:, 4 * h + q, c * 128:(c + 1) * 128],
                                    ident, start=True, stop=True)
                            xt_sb = sb_xt.tile([128, 512], fp16, tag="xts")
                            if c == 3:
                                nc.scalar.copy(xt_sb, xt_ps)
                            else:
                                nc.vector.tensor_copy(xt_sb, xt_ps)
                            nc.tensor.matmul(out_ps, wk_sb[:, c, :], xt_sb,
                                             start=(c == 0), stop=False)
                        # loc term: pa columns in permuted token order
                        # col j=(q,p) -> token 8p + 4h + q of this supertile
                        _sl = pa_sh[:, b, G * 1024 + 4 * h:]
                        pa_slice = bass.AP(tensor=_sl.tensor, offset=_sl.offset,
                                           ap=[_sl.ap[0], [1, 4], [8, 128]])
                        nc.tensor.matmul(out_ps, ww_sb, pa_slice,
                                         start=False, stop=True)

                        tanh_sb = sb_tanh.tile([ATTN, 512], fp16, tag="tanh")
                        nc.scalar.activation(tanh_sb, out_ps, AF.Tanh,
                                             bias=qt_sb[:, b:b + 1])

                        e_ps = ps_o.tile([1, 512], f32, tag="o")
                        nc.tensor.matmul(e_ps, v_sb, tanh_sb, start=True,
                                         stop=True)
                        exp_g = sb_xt.tile([1, 512], f32, tag="expg")
                        nc.scalar.activation(exp_g, e_ps, AF.Exp)
                        # ACT cannot write at partition base b; HWDGE DMA can
                        # (sync ring: keeps it off the SWDGE enc stream).
                        nc.sync.dma_start(
                            exp_sb[b:b + 1,
                                   G * 1024 + h * 512:G * 1024 + (h + 1) * 512],
                            exp_g)

            # softmax normalization over T, batched across the 8 rows
            sums = sb_sm.tile([BL, 1], f32, tag="sums")
            nc.vector.reduce_sum(sums, exp_sb, axis=mybir.AxisListType.X)
            inv = sb_sm.tile([BL, 1], f32, tag="inv")
            nc.vector.reciprocal(inv, sums)
            o_sb = sb_sm.tile([BL, T], f32, tag="osb")
            # un-permute: natural token t = G*1024 + 8p + 4h + q reads storage
            # index G*1024 + h*512 + q*128 + p
            _e = exp_sb[:, :]
            exp_perm = bass.AP(
                tensor=_e.tensor, offset=_e.offset,
                ap=[_e.ap[0], [1024, NSUP], [1, 128], [512, 2], [128, 4]])
            nc.vector.tensor_scalar_mul(o_sb, exp_perm, inv)
            nc.sync.dma_start(out_d, o_sb)

    nc.compile()
    return nc


_cache = {}


def _get(reps: int = 1):
    if reps not in _cache:
        _cache[reps] = build(reps)
    return _cache[reps]


def _in_maps(inputs):
    enc = np.ascontiguousarray(np.asarray(inputs["encoder_outputs"], dtype=np.float32))
    dh = np.ascontiguousarray(np.asarray(inputs["decoder_hidden"], dtype=np.float32))
    pa = np.ascontiguousarray(np.asarray(inputs["prev_attention"], dtype=np.float32))
    rep = {k: np.ascontiguousarray(np.asarray(inputs[k], dtype=np.float32))
           for k in ("Wq", "Wk", "conv_w", "conv_b", "Wl", "v")}
    maps = []
    for i in range(N_CORES):
        s = slice(i * BL, (i + 1) * BL)
        maps.append({"encoder_outputs": enc[s], "decoder_hidden": dh[s],
                     "prev_attention": pa[s], **rep})
    return maps


def kernel(**inputs) -> np.ndarray:
    nc = _get(1)
    res = run_bass_kernel_spmd(nc, _in_maps(inputs), list(range(N_CORES)))
    return np.concatenate([res.results[i]["out"] for i in range(N_CORES)],
                          axis=0).astype(np.float32)


if __name__ == "__main__":
    rng = np.random.default_rng(0)
    ins = {
        "encoder_outputs": rng.standard_normal((B, T, ENC_DIM), dtype=np.float32),
        "decoder_hidden": rng.standard_normal((B, Q_DIM), dtype=np.float32),
        "prev_attention": rng.random((B, T), dtype=np.float32),
        "Wq": (rng.standard_normal((Q_DIM, ATTN), dtype=np.float32) / np.sqrt(Q_DIM)),
        "Wk": (rng.standard_normal((ENC_DIM, ATTN), dtype=np.float32) / np.sqrt(ENC_DIM)),
        "conv_w": (rng.standard_normal((CH, 1, KS), dtype=np.float32) / np.sqrt(KS)),
        "conv_b": np.zeros(CH, dtype=np.float32),
        "Wl": (rng.standard_normal((CH, ATTN), dtype=np.float32) / np.sqrt(CH)),
        "v": (rng.standard_normal(ATTN, dtype=np.float32) / np.sqrt(ATTN)),
    }
    out = kernel(**ins)
    print("kernel output", out.shape, out.dtype, "row sums ~1:",
          np.allclose(out.sum(axis=1), 1.0, atol=1e-3))

